# revision 18
# baseline (speedup 1.0000x reference)
"""Trainium2 Bass kernel for nn_Attn (additive attention energies + softmax).

Reference computation (per batch b):
    c[g]      = sum_h Wh[g,h] * hidden[b,h] + bias[g]          (Wh = W[:, :H])
    pre[t,g]  = tanh(c[g] + sum_h enc[b,t,h] * We[g,h])        (We = W[:, H:])
    en[t]     = sum_g pre[t,g] * v[g]
    out[b,t]  = softmax_t(en)

Shapes: H=1024, B=32, T=1024.  Sharding: data-parallel over batch across 8
cores (4 batches per core); W/bias/v replicated.

v3 strategy (vs the v2 baseline at ~370 us cost-model / ~397 us HW): in v2
every byte of enc and W flowed DRAM->DRAM through a SWDGE cast and then back
through the DMA xbar transposer; in the TRN2 cost model all DMA transfers
serialize on one ~360 GB/s device and the xbar is charged ~14 ns per 32x32
tile (~146 GB/s), so the DMA device was busy ~126 us and, with the
dependency chains, the kernel ran ~3.3x over the 109 us fp16 PE floor.
v3 keeps the DMA device to plain HBM->SBUF loads (~74 us) and moves all
reshaping on-chip:
  - enc: fp32 [128t x 4to x 1024h] loads, fp32->fp16 cast on the (otherwise
    idle) Pool engine, 128x128 PE transposes (fp16, 1 cyc/row) into fp16
    PSUM, ACT copies PSUM->SBUF interleaved with the tanh stream.
  - W: fp32 [128g x 2048c] row loads, fp16 cast on ACT, PE transposes into
    fp16 PSUM, DVE copies into one [128h' x 16hc x 1024g] wT tile.
  - hidden: tiny PE transposes via an identity sub-AP.
  - MM1 (fp16): psum[g=128, t=512] += wT[:,8+hc,gi]^T @ encT[hc] over 8
    h-chunks; ACT fuses (+c bias, tanh) PSUM->SBUF fp16.
  - mm2: v-scale + fp16 tree-add on DVE, one ones-vector matmul per round.
  - softmax over t on [4,1024] after a DRAM gather bounce (direct 3D
    scatter DMA returns garbage on HW; SBUF->SBUF DMA is a deadlock hazard
    alongside xbar use -- and with no xbar left we still keep the proven
    DRAM bounce).
The per-round software pipeline runs two rounds ahead (DMA r+2, cast r+2 on
Pool, PE transposes r+1 interleaved between MM1 groups of round r) so the
PE never goes idle: idle resets the HAM clock ramp (1.2 GHz for 3 us).
"""

import numpy as np

try:
    import concourse  # noqa: F401
except ImportError:  # pragma: no cover
    import sys

    sys.path.insert(0, "/opt/trn_rl_repo")

import concourse.bass as bass  # noqa: E402
import concourse.mybir as mybir  # noqa: E402
import concourse.tile as tile  # noqa: E402
from concourse import bacc  # noqa: E402
from concourse.bass_utils import run_bass_kernel_spmd  # noqa: E402
from concourse.masks import make_identity  # noqa: E402

H = 1024
B = 32
T = 1024
N_CORES = 8
B_LOC = B // N_CORES  # 4 batches per core

F32 = mybir.dt.float32
F16 = mybir.dt.float16
AFT = mybir.ActivationFunctionType

HC = H // 128  # 8 h-chunks
GC = H // 128  # 8 g-chunks
TCH = 512  # t-chunk (PSUM one-bank limit at fp32)
TO = TCH // 128  # 4 partition-blocks per t-chunk
N_ROUNDS = B_LOC * (T // TCH)  # 8 rounds of (batch, t-chunk)


def build_bass(
    repeat_n=None, mm1_dt=F16, unroll=2, ablate=None, cast="pool", etcopy="act"
):
    """Build the per-core Bass program.

    repeat_n: if set, wrap the main phase in a hardware For_i loop that
    re-executes it repeat_n times (timing mode only; results stale after
    iteration 1).  Must be divisible by `unroll`.
    """
    nc = bacc.Bacc("TRN2", target_bir_lowering=False, debug=False)

    enc = nc.dram_tensor("enc", [B_LOC, T, H], F32, kind="ExternalInput").ap()
    hid = nc.dram_tensor("hid", [B_LOC, H], F32, kind="ExternalInput").ap()
    w = nc.dram_tensor("w", [H, 2 * H], F32, kind="ExternalInput").ap()
    bias = nc.dram_tensor("bias", [H], F32, kind="ExternalInput").ap()
    v = nc.dram_tensor("v", [H], F32, kind="ExternalInput").ap()
    out = nc.dram_tensor("out", [B_LOC, T], F32, kind="ExternalOutput").ap()

    with tile.TileContext(nc) as tc:
        ctx_pools = []

        def pool(name, bufs, space="SBUF"):
            p = tc.tile_pool(name=name, bufs=bufs, space=space)
            ctx_pools.append(p)
            return p.__enter__()

        consts = pool("consts", 1)
        wp32 = pool("wp32", 4)
        wp16 = pool("wp16", 3)
        wtp = pool("wtp", 1)
        cpool = pool("cpool", 2)
        ep32 = pool("ep32", 2)
        ep16 = pool("ep16", 2)
        encTp = pool("encT", 16)
        tanhp = pool("tanh", 12)
        vredp = pool("vred", 10)
        esb = pool("esb", 1)
        # PSUM: 8 banks; 2 (mm) + 2 (tr) + 2 (w/hts) + 1 (c) + 1 (en).
        ps_mm = pool("ps_mm", 2, space="PSUM")
        ps_tr = pool("ps_tr", 2, space="PSUM")
        ps_w = pool("ps_w", 2, space="PSUM")
        ps_c = pool("ps_c", 1, space="PSUM")
        ps_en = pool("ps_en", 1, space="PSUM")

        # ---- constants (outside the timing loop, matching v2 accounting) ----
        bias_sb = consts.tile([128, GC], F32, tag="bias_sb", name="bias_sb")
        nc.sync.dma_start(bias_sb[:], bias.rearrange("(o p) -> p o", p=128))
        vf = consts.tile([128, GC], F32, tag="vf", name="vf")
        nc.sync.dma_start(vf[:], v.rearrange("(o p) -> p o", p=128))
        v16 = consts.tile([128, GC], mm1_dt, tag="v16", name="v16")
        nc.vector.tensor_copy(v16[:], vf[:])
        ones16 = consts.tile([128, 1], mm1_dt, tag="ones16", name="ones16")
        nc.vector.memset(ones16[:], 1.0)
        id16 = consts.tile([128, 128], mm1_dt, tag="id16", name="id16")
        make_identity(nc, id16[:])

        # hidden -> fp16, 16-partition-padded so the PE transpose can use an
        # identity sub-AP (the zero padding lands in hts columns the
        # c-matmuls never read).
        hf = consts.tile([B_LOC, H], F32, tag="hf", name="hf")
        nc.sync.dma_start(hf[:], hid)
        h16 = consts.tile([16, H], mm1_dt, tag="h16", name="h16")
        nc.vector.memset(h16[:], 0.0)
        nc.vector.tensor_copy(h16[:B_LOC, :], hf[:])

        def emit_main():
            # Per-body tiles (parity-alternating via pool bufs in timing
            # mode, so consecutive For_i bodies pipeline).
            # wT[j, cc, g] == W[g, cc*128 + j] after the transpose phase.
            wT = wtp.tile([128, 2 * HC, H], mm1_dt, tag="wT", name="wT")
            hts_sb = cpool.tile([128, HC, 16], mm1_dt, tag="hts", name="hts")
            c_sb = [
                cpool.tile([128, B_LOC], F32, tag=f"c{gi}", name=f"c{gi}")
                for gi in range(GC)
            ]
            e_stack = esb.tile([1, N_ROUNDS * TCH], F32, tag="e_stack", name="e_stack")

            def emit_enc_load(r):
                """HWDGE fp32 load of one (batch, t-chunk) of enc to SBUF.
                Split per to-block so the Pool casts start as soon as each
                quarter lands instead of waiting for the full 2 MB."""
                b, tcx = divmod(r, T // TCH)
                t0 = tcx * TCH
                e32 = ep32.tile([128, TO, H], F32, tag="e32", name="e32")
                src = enc[b, t0 : t0 + TCH, :].rearrange(
                    "(to p) h -> p to h", p=128
                )
                for to in range(TO):
                    nc.sync.dma_start(e32[:, to, :], src[:, to, :])
                return e32

            def emit_enc_cast(e32):
                """fp32 -> fp16, one op per to-block.  cast='pool' puts all
                four on the (otherwise idle) Pool engine; 'vec' alternates
                DVE/ACT; 'mix' spreads Pool/Pool/DVE/ACT."""
                e16 = ep16.tile([128, TO, H], mm1_dt, tag="e16", name="e16")
                engs = {
                    "pool": [nc.gpsimd] * 4,
                    "vec": [nc.vector, nc.scalar, nc.vector, nc.scalar],
                    "mix": [nc.gpsimd, nc.gpsimd, nc.vector, nc.scalar],
                }[cast]
                for to in range(TO):
                    eng = engs[to]
                    if eng is nc.scalar:
                        eng.copy(e16[:, to, :], e32[:, to, :])
                    else:
                        eng.tensor_copy(e16[:, to, :], e32[:, to, :])
                return e16

            def emit_enc_tr(e16, hc):
                """PE-transpose h-chunk hc of a cast t-chunk into SBUF."""
                pt = ps_tr.tile([128, TCH], mm1_dt, tag="ps_tr", name="ps_tr")
                for to in range(TO):
                    nc.tensor.transpose(
                        pt[:, 128 * to : 128 * (to + 1)],
                        e16[:, to, 128 * hc : 128 * (hc + 1)],
                        id16[:],
                    )
                et = encTp.tile([128, TCH], mm1_dt, tag="encT", name="encT")
                if etcopy == "act":
                    nc.scalar.copy(et[:], pt[:])
                else:
                    nc.vector.tensor_copy(et[:], pt[:])
                return et

            def emit_mm2(r, tanh_tiles, on_pe=False):
                """v-reduction over g for round r's tanh tiles.  Deferred past
                the next round's early MM1 groups so the PE never waits.
                on_pe: 8 accumulated v-column matmuls instead of the DVE tree
                -- costlier on the PE, used for the last round where the PE is
                otherwise done and the DVE tree would sit on the kernel tail."""
                pen = ps_en.tile([1, TCH], F32, tag="ps_en", name="ps_en")
                if on_pe:
                    for gi in range(GC):
                        nc.tensor.matmul(
                            pen[:],
                            v16[:, gi : gi + 1],
                            tanh_tiles[gi][:],
                            start=(gi == 0),
                            stop=(gi == GC - 1),
                        )
                else:
                    lvl = []
                    for gi in range(GC):
                        m = vredp.tile([128, TCH], F16, tag="vmul", name="vmul")
                        nc.vector.tensor_scalar_mul(
                            m[:], tanh_tiles[gi][:], vf[:, gi : gi + 1]
                        )
                        lvl.append(m)
                    # in-place pairwise tree-add: (0,1)(2,3)... -> lvl[0]
                    stride = 1
                    while stride < GC:
                        for i in range(0, GC, 2 * stride):
                            nc.vector.tensor_add(
                                lvl[i][:], lvl[i][:], lvl[i + stride][:]
                            )
                        stride *= 2
                    nc.tensor.matmul(
                        pen[:], ones16[:], lvl[0][:], start=True, stop=True
                    )
                nc.scalar.copy(e_stack[:, TCH * r : TCH * (r + 1)], pen[:])

            def emit_softmax(b):
                """softmax over t for one batch, straight off its contiguous
                [1, T] segment of e_stack (partition 0).  Emitted right after
                that batch's second emit_mm2 so only batch 3's softmax is
                kernel tail; the rest hide under later rounds."""
                seg = e_stack[:, T * b : T * (b + 1)]
                mx = esb.tile([1, 1], F32, tag="mx", name="mx")
                nc.vector.reduce_max(mx[:], seg, axis=mybir.AxisListType.X)
                nmx = esb.tile([1, 1], F32, tag="nmx", name="nmx")
                nc.vector.tensor_scalar_mul(nmx[:], mx[:], -1.0)
                ex = esb.tile([1, T], F32, tag="ex", name="ex")
                sm = esb.tile([1, 1], F32, tag="sm", name="sm")
                nc.scalar.activation(
                    ex[:], seg, AFT.Exp, bias=nmx[:], scale=1.0, accum_out=sm[:]
                )
                rs = esb.tile([1, 1], F32, tag="rs", name="rs")
                nc.vector.reciprocal(rs[:], sm[:])
                osb = esb.tile([1, T], F32, tag="osb", name="osb")
                nc.vector.tensor_scalar_mul(osb[:], ex[:], rs[:])
                nc.sync.dma_start(out[b : b + 1, :], osb[:])

            def drain_mm2(pending):
                r, tanh_tiles = pending
                emit_mm2(r, tanh_tiles, on_pe=(r == N_ROUNDS - 1))
                if r % 2 == 1:
                    emit_softmax(r // 2)

            def emit_w_block(go, half, w32):
                """ACT cast of one W half-row, PE transposes of its 8 column
                chunks (4 per fp16 PSUM tile), DVE copies into wT."""
                w16 = wp16.tile([128, H], mm1_dt, tag="w16", name="w16")
                nc.scalar.copy(w16[:], w32[:])
                for q in range(2):
                    ccq = 2 * half + q
                    pw = ps_w.tile([128, 512], mm1_dt, tag="ps_w", name="ps_w")
                    for j in range(4):
                        cc = 4 * q + j
                        nc.tensor.transpose(
                            pw[:, 128 * j : 128 * (j + 1)],
                            w16[:, 128 * cc : 128 * (cc + 1)],
                            id16[:],
                        )
                    nc.vector.tensor_copy(
                        wT[:, 4 * ccq : 4 * (ccq + 1), 128 * go : 128 * (go + 1)],
                        pw[:].rearrange("p (c g) -> p c g", c=4),
                    )

            def emit_c(gi):
                """c[g, b] = sum_h Wh[g,h] h[b,h] + bias[g] for g-chunk gi."""
                pc = ps_c.tile([128, B_LOC], F32, tag="ps_c", name="ps_c")
                for hc in range(HC):
                    nc.tensor.matmul(
                        pc[:],
                        wT[:, hc, 128 * gi : 128 * (gi + 1)],
                        hts_sb[:, hc, :B_LOC],
                        start=(hc == 0),
                        stop=(hc == HC - 1),
                    )
                nc.vector.tensor_scalar_add(c_sb[gi][:], pc[:], bias_sb[:, gi : gi + 1])

            def emit_mm1_group(r, gi, encT_list, tanh_tiles):
                b = r // (T // TCH)
                pm = ps_mm.tile([128, TCH], F32, tag="ps_mm", name="ps_mm")
                for hc in range(HC):
                    nc.tensor.matmul(
                        pm[:],
                        wT[:, HC + hc, 128 * gi : 128 * (gi + 1)],
                        encT_list[hc][:],
                        start=(hc == 0),
                        stop=(hc == HC - 1),
                    )
                th = tanhp.tile([128, TCH], F16, tag="tanh", name="tanh")
                nc.scalar.activation(
                    th[:], pm[:], AFT.Tanh, bias=c_sb[gi][:, b : b + 1], scale=1.0
                )
                tanh_tiles.append(th)

            # ---- prologue: W / hidden / c setup interleaved with round 0 ----
            # DMA order: enc chunk 0, W rows for go 0, enc chunk 1, the rest
            # of W.  W rides in [128, 1024] half-rows through a 4-slot
            # rotating pool so the whole 8.4 MB never sits in SBUF at once;
            # the cast keeps pace with the rotation so the staggered slot-WAR
            # waits on the SP queue never delay the later enc-load issues.
            def w_load(go, half):
                w32 = wp32.tile([128, H], F32, tag="w32", name="w32")
                nc.sync.dma_start(
                    w32[:],
                    w[128 * go : 128 * (go + 1), H * half : H * (half + 1)],
                )
                return w32

            e32_pend = [emit_enc_load(0)]
            w32s = [w_load(0, 0), w_load(0, 1)]
            e32_pend.append(emit_enc_load(1))
            for go in range(1, GC):
                w32s += [w_load(go, 0), w_load(go, 1)]
            e16_pend = [emit_enc_cast(e32_pend[0]), emit_enc_cast(e32_pend[1])]

            # hidden transposes first on the PE queue (depend only on consts):
            # [16, 128] -> [128, 16] per h-chunk.
            ph = ps_w.tile([128, 512], mm1_dt, tag="ps_w", name="ps_w")
            for hc in range(HC):
                nc.tensor.transpose(
                    ph[:, 16 * hc : 16 * (hc + 1)],
                    h16[:, 128 * hc : 128 * (hc + 1)],
                    id16[:16, :16],
                )
            nc.vector.tensor_copy(
                hts_sb[:], ph[:, : 16 * HC].rearrange("p (c b) -> p c b", c=HC)
            )

            # Round 0's MM1 group gi rides right behind W row-block gi+1's
            # transposes so the PE chews on MM1 while W streams in.
            emit_w_block(0, 0, w32s[0])
            emit_w_block(0, 1, w32s[1])
            emit_c(0)
            encT_cur = [emit_enc_tr(e16_pend[0], hc) for hc in range(HC)]

            if ablate == "dma_only":
                for r in range(2, N_ROUNDS):
                    emit_enc_load(r)
                nc.sync.dma_start(out, hf[:])
                return

            tanh0 = []
            encT_next = [None] * HC
            for go in range(1, GC):
                emit_w_block(go, 0, w32s[2 * go])
                emit_w_block(go, 1, w32s[2 * go + 1])
                emit_c(go)
                gi = go - 1
                emit_mm1_group(0, gi, encT_cur, tanh0)
                if ablate != "pe_only":
                    if gi == 0:
                        e32_pend.append(emit_enc_load(2))
                    if gi == 1:
                        e16_pend.append(emit_enc_cast(e32_pend[2]))
                    encT_next[gi] = emit_enc_tr(e16_pend[1], gi)
            emit_mm1_group(0, GC - 1, encT_cur, tanh0)
            if ablate != "pe_only":
                encT_next[GC - 1] = emit_enc_tr(e16_pend[1], GC - 1)
                encT_cur = encT_next
            pending_mm2 = (0, tanh0)

            # ---- rounds 1..N-1 ----
            for r in range(1, N_ROUNDS):
                tanh_tiles = []
                encT_next = [None] * HC
                for gi in range(GC):
                    emit_mm1_group(r, gi, encT_cur, tanh_tiles)
                    if ablate != "pe_only":
                        if gi == 0 and r + 2 < N_ROUNDS:
                            e32_pend.append(emit_enc_load(r + 2))
                        if gi == 1 and r + 2 < N_ROUNDS:
                            e16_pend.append(emit_enc_cast(e32_pend[r + 2]))
                        # round r+1's transposes ride between MM1 groups.
                        if r + 1 < N_ROUNDS:
                            encT_next[gi] = emit_enc_tr(e16_pend[r + 1], gi)
                    if gi == 3 and pending_mm2 is not None:
                        drain_mm2(pending_mm2)
                        pending_mm2 = None
                pending_mm2 = (r, tanh_tiles)
                if ablate != "pe_only" and r + 1 < N_ROUNDS:
                    encT_cur = encT_next
            drain_mm2(pending_mm2)

        if repeat_n:
            assert repeat_n % unroll == 0, (repeat_n, unroll)
            with tc.For_i(0, repeat_n // unroll, 1):
                for _ in range(unroll):
                    emit_main()
        else:
            emit_main()

        for p in reversed(ctx_pools):
            p.__exit__(None, None, None)

    nc.compile()
    return nc


_NC = None


def _get_nc():
    global _NC
    if _NC is None:
        _NC = build_bass()
    return _NC


def kernel(hidden, encoder_outputs, W, b, v):
    nc = _get_nc()
    hidden = np.asarray(hidden, dtype=np.float32)
    encoder_outputs = np.asarray(encoder_outputs, dtype=np.float32)
    W = np.asarray(W, dtype=np.float32)
    b = np.asarray(b, dtype=np.float32)
    v = np.asarray(v, dtype=np.float32)
    hid = hidden[0]  # [B, H]
    in_maps = []
    for i in range(N_CORES):
        s = slice(B_LOC * i, B_LOC * (i + 1))
        in_maps.append(
            {
                "enc": np.ascontiguousarray(encoder_outputs[s]),
                "hid": np.ascontiguousarray(hid[s]),
                "w": W,
                "bias": b,
                "v": v,
            }
        )
    res = run_bass_kernel_spmd(nc, in_maps, core_ids=list(range(N_CORES)))
    full = np.concatenate([res.results[i]["out"] for i in range(N_CORES)], axis=0)
    return full[:, None, :].astype(np.float32)


# revision 23
# speedup vs baseline: 1.1056x; 1.1056x over previous
"""Trainium2 Bass kernel for nn_Attn (additive attention energies + softmax).

Reference computation (per batch b):
    c[g]      = sum_h Wh[g,h] * hidden[b,h] + bias[g]          (Wh = W[:, :H])
    pre[t,g]  = tanh(c[g] + sum_h enc[b,t,h] * We[g,h])        (We = W[:, H:])
    en[t]     = sum_g pre[t,g] * v[g]
    out[b,t]  = softmax_t(en)

Shapes: H=1024, B=32, T=1024.  Sharding: data-parallel over batch across 8
cores (4 batches per core); W/bias/v replicated.

v3 strategy (vs the v2 baseline at ~397 us harness / ~370 us cost-model):
in v2 every byte of enc and W flowed DRAM->DRAM through a SWDGE cast and
then back through the DMA xbar transposer; all DMA transfers contend on the
same ~360 GB/s device, so the DMA side was busy ~126 us and, with the
dependency chains, the kernel ran ~3.3x over the 109 us fp16 PE floor.
v3 keeps the DMA device to plain HBM->SBUF loads (~74 us, 25.3 MB/call)
and moves all reshaping on-chip:
  - enc: fp32 [128t x 4to x 1024h] loads (split per to-block), fp32->fp16
    cast on the (otherwise idle) Pool engine, 128x128 PE transposes (fp16,
    1 cyc/row) into fp16 PSUM, ACT copies PSUM->SBUF interleaved with the
    tanh stream.  (trans='xbar' keeps v2's SWDGE-cast + xbar path instead;
    measured equal at sustained load, more HBM bytes -- not default.)
  - W: fp32 [128g x 1024c] half-row loads through a 4-slot pool, fp16 cast
    on ACT, PE transposes into fp16 PSUM, DVE copies into one
    [128h' x 16hc x 1024g] wT tile.  Round-0 MM1 groups are emitted
    interleaved with the W row-blocks so the PE chews on MM1 while W
    streams in (the prologue is DMA-landing-bound).
  - hidden: tiny PE transposes via an identity sub-AP.
  - MM1 (fp16): psum[g=128, t=512] += wT[:,8+hc,gi]^T @ encT[hc] over 8
    h-chunks; ACT fuses (+c bias, tanh) PSUM->SBUF fp16.
  - mm2: v-scale + in-place fp16 tree-add on DVE + one ones-vector matmul
    per round; the last round instead uses 8 accumulated v-column matmuls
    on the (by then idle) PE so the DVE tree is off the kernel tail.
  - softmax per batch straight off its contiguous [1, T] slice of e_stack
    (energies land round-major, so each batch's two 512-chunks are already
    adjacent) -- no DRAM gather bounce; batches 0-2 hide under later
    rounds, only batch 3's softmax is kernel tail.
The per-round software pipeline runs two rounds ahead (DMA r+2 and Pool
cast r+2 during round r, PE transposes for r+1 interleaved between MM1
groups of round r) so the PE never goes idle: idle resets the HAM clock
ramp (1.2 GHz for 3 us).
fp8 (DoubleRow) MM1 was evaluated and rejected: e4m3 quantization noise of
enc/We (~3.6% RMS) through the 1024-term v-reduction gives ~0.3 softmax
rel err vs the 2e-2 gate; fp16 measures 3.2e-3.
Measured (test.py slope, K=2..20002): v2 396.9 us -> v3 ~260-313 us (the
per-iteration rate inflates ~2x between K=2002 and K=20002 for all
variants -- sustained-load effect, MM1 itself stays at full clock by the
mm_half probe; cross-process variance is +-20-50 us).
"""

import numpy as np

try:
    import concourse  # noqa: F401
except ImportError:  # pragma: no cover
    import sys

    sys.path.insert(0, "/opt/trn_rl_repo")

import concourse.bass as bass  # noqa: E402
import concourse.mybir as mybir  # noqa: E402
import concourse.tile as tile  # noqa: E402
from concourse import bacc  # noqa: E402
from concourse.bass_utils import run_bass_kernel_spmd  # noqa: E402
from concourse.masks import make_identity  # noqa: E402

H = 1024
B = 32
T = 1024
N_CORES = 8
B_LOC = B // N_CORES  # 4 batches per core

F32 = mybir.dt.float32
F16 = mybir.dt.float16
AFT = mybir.ActivationFunctionType

HC = H // 128  # 8 h-chunks
GC = H // 128  # 8 g-chunks
TCH = 512  # t-chunk (PSUM one-bank limit at fp32)
TO = TCH // 128  # 4 partition-blocks per t-chunk
N_ROUNDS = B_LOC * (T // TCH)  # 8 rounds of (batch, t-chunk)


def build_bass(
    repeat_n=None,
    mm1_dt=F16,
    unroll=2,
    ablate=None,
    cast="pool",
    etcopy="act",
    trans="pe",
):
    """trans='pe': enc fp32->SBUF, Pool cast, PE transposes (cheap when the
    PE clock holds 2.4 GHz).  trans='xbar': SWDGE fp16 cast DRAM->DRAM plus
    DMA-xbar transposes DRAM->SBUF, keeping the PE on matmuls only (wins when
    sustained load throttles the PE clock).  All xbar transposes stay on the
    single SP ring: concurrent dual-ring xbar transposes corrupt each other
    (HW-verified in v2), and no SBUF->SBUF DMAs may run alongside them."""
    """Build the per-core Bass program.

    repeat_n: if set, wrap the main phase in a hardware For_i loop that
    re-executes it repeat_n times (timing mode only; results stale after
    iteration 1).  Must be divisible by `unroll`.
    """
    nc = bacc.Bacc("TRN2", target_bir_lowering=False, debug=False)

    enc = nc.dram_tensor("enc", [B_LOC, T, H], F32, kind="ExternalInput").ap()
    hid = nc.dram_tensor("hid", [B_LOC, H], F32, kind="ExternalInput").ap()
    w = nc.dram_tensor("w", [H, 2 * H], F32, kind="ExternalInput").ap()
    bias = nc.dram_tensor("bias", [H], F32, kind="ExternalInput").ap()
    v = nc.dram_tensor("v", [H], F32, kind="ExternalInput").ap()
    out = nc.dram_tensor("out", [B_LOC, T], F32, kind="ExternalOutput").ap()

    with tile.TileContext(nc) as tc:
        ctx_pools = []

        def pool(name, bufs, space="SBUF"):
            p = tc.tile_pool(name=name, bufs=bufs, space=space)
            ctx_pools.append(p)
            return p.__enter__()

        consts = pool("consts", 1)
        wp32 = pool("wp32", 4)
        wp16 = pool("wp16", 3)
        wtp = pool("wtp", 1)
        cpool = pool("cpool", 2)
        ep32 = pool("ep32", 2)
        ep16 = pool("ep16", 2)
        encTp = pool("encT", 16)
        tanhp = pool("tanh", 12)
        vredp = pool("vred", 10)
        esb = pool("esb", 1)
        if trans == "xbar":
            edram = pool("edram", 3, space="DRAM")
        # PSUM: 8 banks; 2 (mm) + 2 (tr) + 2 (w/hts) + 1 (c) + 1 (en).
        ps_mm = pool("ps_mm", 2, space="PSUM")
        ps_tr = pool("ps_tr", 2, space="PSUM")
        ps_w = pool("ps_w", 2, space="PSUM")
        ps_c = pool("ps_c", 1, space="PSUM")
        ps_en = pool("ps_en", 1, space="PSUM")

        # ---- constants (outside the timing loop, matching v2 accounting) ----
        bias_sb = consts.tile([128, GC], F32, tag="bias_sb", name="bias_sb")
        nc.sync.dma_start(bias_sb[:], bias.rearrange("(o p) -> p o", p=128))
        vf = consts.tile([128, GC], F32, tag="vf", name="vf")
        nc.sync.dma_start(vf[:], v.rearrange("(o p) -> p o", p=128))
        v16 = consts.tile([128, GC], mm1_dt, tag="v16", name="v16")
        nc.vector.tensor_copy(v16[:], vf[:])
        ones16 = consts.tile([128, 1], mm1_dt, tag="ones16", name="ones16")
        nc.vector.memset(ones16[:], 1.0)
        id16 = consts.tile([128, 128], mm1_dt, tag="id16", name="id16")
        make_identity(nc, id16[:])

        # hidden -> fp16, 16-partition-padded so the PE transpose can use an
        # identity sub-AP (the zero padding lands in hts columns the
        # c-matmuls never read).
        hf = consts.tile([B_LOC, H], F32, tag="hf", name="hf")
        nc.sync.dma_start(hf[:], hid)
        h16 = consts.tile([16, H], mm1_dt, tag="h16", name="h16")
        nc.vector.memset(h16[:], 0.0)
        nc.vector.tensor_copy(h16[:B_LOC, :], hf[:])

        def emit_main():
            # Per-body tiles (parity-alternating via pool bufs in timing
            # mode, so consecutive For_i bodies pipeline).
            # wT[j, cc, g] == W[g, cc*128 + j] after the transpose phase.
            wT = wtp.tile([128, 2 * HC, H], mm1_dt, tag="wT", name="wT")
            hts_sb = cpool.tile([128, HC, 16], mm1_dt, tag="hts", name="hts")
            c_sb = [
                cpool.tile([128, B_LOC], F32, tag=f"c{gi}", name=f"c{gi}")
                for gi in range(GC)
            ]
            e_stack = esb.tile([1, N_ROUNDS * TCH], F32, tag="e_stack", name="e_stack")

            def emit_enc_load(r):
                """Stage one (batch, t-chunk) of enc for transposition.
                trans='pe': HWDGE fp32 load to SBUF, split per to-block so
                the casts start as soon as each quarter lands.
                trans='xbar': SWDGE fp32->fp16 cast DMA into DRAM scratch."""
                b, tcx = divmod(r, T // TCH)
                t0 = tcx * TCH
                if trans == "xbar":
                    scr = edram.tile([TCH, H], mm1_dt, tag="escr", name="escr")
                    nc.gpsimd.dma_start(scr[:], enc[b, t0 : t0 + TCH, :])
                    return scr
                e32 = ep32.tile([128, TO, H], F32, tag="e32", name="e32")
                src = enc[b, t0 : t0 + TCH, :].rearrange(
                    "(to p) h -> p to h", p=128
                )
                for to in range(TO):
                    nc.sync.dma_start(e32[:, to, :], src[:, to, :])
                return e32

            def emit_enc_cast(e32):
                """fp32 -> fp16, one op per to-block.  cast='pool' puts all
                four on the (otherwise idle) Pool engine; 'vec' alternates
                DVE/ACT; 'mix' spreads Pool/Pool/DVE/ACT.  No-op for xbar
                (the cast rode inside the SWDGE DMA)."""
                if trans == "xbar":
                    return e32
                e16 = ep16.tile([128, TO, H], mm1_dt, tag="e16", name="e16")
                engs = {
                    "pool": [nc.gpsimd] * 4,
                    "vec": [nc.vector, nc.scalar, nc.vector, nc.scalar],
                    "mix": [nc.gpsimd, nc.gpsimd, nc.vector, nc.scalar],
                }[cast]
                for to in range(TO):
                    eng = engs[to]
                    if eng is nc.scalar:
                        eng.copy(e16[:, to, :], e32[:, to, :])
                    else:
                        eng.tensor_copy(e16[:, to, :], e32[:, to, :])
                return e16

            def emit_enc_tr(e16, hc):
                """Transpose h-chunk hc of a staged t-chunk into SBUF: PE
                128x128 transposes via PSUM, or one DMA-xbar transpose from
                the fp16 DRAM scratch (SP ring only)."""
                et = encTp.tile([128, TCH], mm1_dt, tag="encT", name="encT")
                if trans == "xbar":
                    nc.sync.dma_start_transpose(
                        et[:], e16[:, 128 * hc : 128 * (hc + 1)]
                    )
                    return et
                pt = ps_tr.tile([128, TCH], mm1_dt, tag="ps_tr", name="ps_tr")
                for to in range(TO):
                    nc.tensor.transpose(
                        pt[:, 128 * to : 128 * (to + 1)],
                        e16[:, to, 128 * hc : 128 * (hc + 1)],
                        id16[:],
                    )
                if etcopy == "act":
                    nc.scalar.copy(et[:], pt[:])
                else:
                    nc.vector.tensor_copy(et[:], pt[:])
                return et

            def emit_mm2(r, tanh_tiles, on_pe=False):
                """v-reduction over g for round r's tanh tiles.  Deferred past
                the next round's early MM1 groups so the PE never waits.
                on_pe: 8 accumulated v-column matmuls instead of the DVE tree
                -- costlier on the PE, used for the last round where the PE is
                otherwise done and the DVE tree would sit on the kernel tail."""
                pen = ps_en.tile([1, TCH], F32, tag="ps_en", name="ps_en")
                if on_pe:
                    for gi in range(GC):
                        nc.tensor.matmul(
                            pen[:],
                            v16[:, gi : gi + 1],
                            tanh_tiles[gi][:],
                            start=(gi == 0),
                            stop=(gi == GC - 1),
                        )
                else:
                    lvl = []
                    for gi in range(GC):
                        m = vredp.tile([128, TCH], F16, tag="vmul", name="vmul")
                        nc.vector.tensor_scalar_mul(
                            m[:], tanh_tiles[gi][:], vf[:, gi : gi + 1]
                        )
                        lvl.append(m)
                    # in-place pairwise tree-add: (0,1)(2,3)... -> lvl[0]
                    stride = 1
                    while stride < GC:
                        for i in range(0, GC, 2 * stride):
                            nc.vector.tensor_add(
                                lvl[i][:], lvl[i][:], lvl[i + stride][:]
                            )
                        stride *= 2
                    nc.tensor.matmul(
                        pen[:], ones16[:], lvl[0][:], start=True, stop=True
                    )
                nc.scalar.copy(e_stack[:, TCH * r : TCH * (r + 1)], pen[:])

            def emit_softmax(b):
                """softmax over t for one batch, straight off its contiguous
                [1, T] segment of e_stack (partition 0).  Emitted right after
                that batch's second emit_mm2 so only batch 3's softmax is
                kernel tail; the rest hide under later rounds."""
                seg = e_stack[:, T * b : T * (b + 1)]
                mx = esb.tile([1, 1], F32, tag="mx", name="mx")
                nc.vector.reduce_max(mx[:], seg, axis=mybir.AxisListType.X)
                nmx = esb.tile([1, 1], F32, tag="nmx", name="nmx")
                nc.vector.tensor_scalar_mul(nmx[:], mx[:], -1.0)
                ex = esb.tile([1, T], F32, tag="ex", name="ex")
                sm = esb.tile([1, 1], F32, tag="sm", name="sm")
                nc.scalar.activation(
                    ex[:], seg, AFT.Exp, bias=nmx[:], scale=1.0, accum_out=sm[:]
                )
                rs = esb.tile([1, 1], F32, tag="rs", name="rs")
                nc.vector.reciprocal(rs[:], sm[:])
                osb = esb.tile([1, T], F32, tag="osb", name="osb")
                nc.vector.tensor_scalar_mul(osb[:], ex[:], rs[:])
                nc.sync.dma_start(out[b : b + 1, :], osb[:])

            def drain_mm2(pending):
                r, tanh_tiles = pending
                emit_mm2(r, tanh_tiles, on_pe=(r == N_ROUNDS - 1))
                if r % 2 == 1:
                    emit_softmax(r // 2)

            def emit_w_block(go, half, w32):
                """ACT cast of one W half-row, PE transposes of its 8 column
                chunks (4 per fp16 PSUM tile), DVE copies into wT."""
                w16 = wp16.tile([128, H], mm1_dt, tag="w16", name="w16")
                nc.scalar.copy(w16[:], w32[:])
                for q in range(2):
                    ccq = 2 * half + q
                    pw = ps_w.tile([128, 512], mm1_dt, tag="ps_w", name="ps_w")
                    for j in range(4):
                        cc = 4 * q + j
                        nc.tensor.transpose(
                            pw[:, 128 * j : 128 * (j + 1)],
                            w16[:, 128 * cc : 128 * (cc + 1)],
                            id16[:],
                        )
                    nc.vector.tensor_copy(
                        wT[:, 4 * ccq : 4 * (ccq + 1), 128 * go : 128 * (go + 1)],
                        pw[:].rearrange("p (c g) -> p c g", c=4),
                    )

            def emit_c(gi):
                """c[g, b] = sum_h Wh[g,h] h[b,h] + bias[g] for g-chunk gi."""
                pc = ps_c.tile([128, B_LOC], F32, tag="ps_c", name="ps_c")
                for hc in range(HC):
                    nc.tensor.matmul(
                        pc[:],
                        wT[:, hc, 128 * gi : 128 * (gi + 1)],
                        hts_sb[:, hc, :B_LOC],
                        start=(hc == 0),
                        stop=(hc == HC - 1),
                    )
                nc.vector.tensor_scalar_add(c_sb[gi][:], pc[:], bias_sb[:, gi : gi + 1])

            def emit_mm1_group(r, gi, encT_list, tanh_tiles):
                b = r // (T // TCH)
                pm = ps_mm.tile([128, TCH], F32, tag="ps_mm", name="ps_mm")
                for hc in range(HC):
                    nc.tensor.matmul(
                        pm[:],
                        wT[:, HC + hc, 128 * gi : 128 * (gi + 1)],
                        encT_list[hc][:],
                        start=(hc == 0),
                        stop=(hc == HC - 1),
                    )
                th = tanhp.tile([128, TCH], F16, tag="tanh", name="tanh")
                nc.scalar.activation(
                    th[:], pm[:], AFT.Tanh, bias=c_sb[gi][:, b : b + 1], scale=1.0
                )
                tanh_tiles.append(th)

            # ---- prologue: W / hidden / c setup interleaved with round 0 ----
            # DMA order: enc chunk 0, W rows for go 0, enc chunk 1, the rest
            # of W.  W rides in [128, 1024] half-rows through a 4-slot
            # rotating pool so the whole 8.4 MB never sits in SBUF at once;
            # the cast keeps pace with the rotation so the staggered slot-WAR
            # waits on the SP queue never delay the later enc-load issues.
            def w_load(go, half):
                w32 = wp32.tile([128, H], F32, tag="w32", name="w32")
                nc.sync.dma_start(
                    w32[:],
                    w[128 * go : 128 * (go + 1), H * half : H * (half + 1)],
                )
                return w32

            e32_pend = [emit_enc_load(0)]
            w32s = [w_load(0, 0), w_load(0, 1)]
            e32_pend.append(emit_enc_load(1))
            for go in range(1, GC):
                w32s += [w_load(go, 0), w_load(go, 1)]
            e16_pend = [emit_enc_cast(e32_pend[0]), emit_enc_cast(e32_pend[1])]

            # hidden transposes first on the PE queue (depend only on consts):
            # [16, 128] -> [128, 16] per h-chunk.
            ph = ps_w.tile([128, 512], mm1_dt, tag="ps_w", name="ps_w")
            for hc in range(HC):
                nc.tensor.transpose(
                    ph[:, 16 * hc : 16 * (hc + 1)],
                    h16[:, 128 * hc : 128 * (hc + 1)],
                    id16[:16, :16],
                )
            nc.vector.tensor_copy(
                hts_sb[:], ph[:, : 16 * HC].rearrange("p (c b) -> p c b", c=HC)
            )

            # Round 0's MM1 group gi rides right behind W row-block gi+1's
            # transposes so the PE chews on MM1 while W streams in.
            emit_w_block(0, 0, w32s[0])
            emit_w_block(0, 1, w32s[1])
            emit_c(0)
            encT_cur = [emit_enc_tr(e16_pend[0], hc) for hc in range(HC)]

            if ablate == "dma_only":
                for r in range(2, N_ROUNDS):
                    emit_enc_load(r)
                nc.sync.dma_start(out, hf[:])
                return

            tanh0 = []
            encT_next = [None] * HC
            for go in range(1, GC):
                emit_w_block(go, 0, w32s[2 * go])
                emit_w_block(go, 1, w32s[2 * go + 1])
                emit_c(go)
                gi = go - 1
                emit_mm1_group(0, gi, encT_cur, tanh0)
                if ablate != "pe_only":
                    if gi == 0:
                        e32_pend.append(emit_enc_load(2))
                    if gi == 1:
                        e16_pend.append(emit_enc_cast(e32_pend[2]))
                    encT_next[gi] = emit_enc_tr(e16_pend[1], gi)
            emit_mm1_group(0, GC - 1, encT_cur, tanh0)
            if ablate != "pe_only":
                encT_next[GC - 1] = emit_enc_tr(e16_pend[1], GC - 1)
                encT_cur = encT_next
            pending_mm2 = (0, tanh0)

            # ---- rounds 1..N-1 ----
            for r in range(1, N_ROUNDS):
                tanh_tiles = []
                encT_next = [None] * HC
                for gi in range(GC):
                    if ablate == "mm_half" and gi % 2:
                        # timing probe: half the MM1 groups (results garbage)
                        tanh_tiles.append(tanh_tiles[-1])
                    else:
                        emit_mm1_group(r, gi, encT_cur, tanh_tiles)
                    if ablate != "pe_only":
                        if gi == 0 and r + 2 < N_ROUNDS:
                            e32_pend.append(emit_enc_load(r + 2))
                        if gi == 1 and r + 2 < N_ROUNDS:
                            e16_pend.append(emit_enc_cast(e32_pend[r + 2]))
                        # round r+1's transposes ride between MM1 groups.
                        if r + 1 < N_ROUNDS:
                            encT_next[gi] = emit_enc_tr(e16_pend[r + 1], gi)
                    if gi == 3 and pending_mm2 is not None:
                        drain_mm2(pending_mm2)
                        pending_mm2 = None
                pending_mm2 = (r, tanh_tiles)
                if ablate != "pe_only" and r + 1 < N_ROUNDS:
                    encT_cur = encT_next
            drain_mm2(pending_mm2)

        if repeat_n:
            assert repeat_n % unroll == 0, (repeat_n, unroll)
            with tc.For_i(0, repeat_n // unroll, 1):
                for _ in range(unroll):
                    emit_main()
        else:
            emit_main()

        for p in reversed(ctx_pools):
            p.__exit__(None, None, None)

    nc.compile()
    return nc


_NC = None


def _get_nc():
    global _NC
    if _NC is None:
        _NC = build_bass()
    return _NC


def kernel(hidden, encoder_outputs, W, b, v):
    nc = _get_nc()
    hidden = np.asarray(hidden, dtype=np.float32)
    encoder_outputs = np.asarray(encoder_outputs, dtype=np.float32)
    W = np.asarray(W, dtype=np.float32)
    b = np.asarray(b, dtype=np.float32)
    v = np.asarray(v, dtype=np.float32)
    hid = hidden[0]  # [B, H]
    in_maps = []
    for i in range(N_CORES):
        s = slice(B_LOC * i, B_LOC * (i + 1))
        in_maps.append(
            {
                "enc": np.ascontiguousarray(encoder_outputs[s]),
                "hid": np.ascontiguousarray(hid[s]),
                "w": W,
                "bias": b,
                "v": v,
            }
        )
    res = run_bass_kernel_spmd(nc, in_maps, core_ids=list(range(N_CORES)))
    full = np.concatenate([res.results[i]["out"] for i in range(N_CORES)], axis=0)
    return full[:, None, :].astype(np.float32)
